# revision 1
# baseline (speedup 1.0000x reference)
"""MoE routed-MLP (GPTNeoX) Trainium2 kernel.

Expert-parallel over 8 NeuronCores: core e holds expert e's weights.
Host computes the (tiny) router + top-2 dispatch, gathers each expert's
tokens into a padded batch, and scatter-adds the weighted expert outputs
back. Each core runs the same SPMD Bass program:

    hT[f, c] = gelu( sum_k w1[k, f] * xT[k, c] + b1[f] )      (f on partitions)
    yT[h, c] = sum_f w2[f, h] * hT[f, c] + b2[h]              (h on partitions)

i.e. both matmuls keep the weights as the stationary operand so the
intermediate never needs an on-chip transpose. All DMAs are contiguous:
the host pre-arranges every operand into [128, ko, free] layout.
"""

import numpy as np

import concourse.bass as bass  # noqa: F401  (bass types used via tile/bacc)
import concourse.mybir as mybir
import concourse.tile as tile
from concourse import bacc
from concourse.bass_utils import run_bass_kernel_spmd

H = 1024
F = 4096
E = 8
NCORES = 8
P = 128
KO = H // P  # 8   k-chunks for the H contraction
FO = F // P  # 32  f-tiles
HO = H // P  # 8   h-tiles

# "fp32r" | "fp32" | "bf16"
KERNEL_DTYPE = "fp32r"

_nc_cache = {}


def _chunks(C):
    """Split [0, C) into column chunks, each a multiple of 64 and <= 512.

    For C >= 512 all chunks land in [256, 512] which keeps fp32r matmuls
    at full rate (moving dim >= 256).
    """
    n = (C + 511) // 512
    base = (C // n) // 64 * 64
    widths = [base] * n
    rem = C - base * n
    i = 0
    while rem > 0:
        widths[i] += 64
        rem -= 64
        i = (i + 1) % n
    out, off = [], 0
    for w in widths:
        out.append((off, w))
        off += w
    return out


def _build(C, dt_tag):
    f32 = mybir.dt.float32
    dt_in = {
        "bf16": mybir.dt.bfloat16,
        "fp32r": mybir.dt.float32r,
        "fp32": f32,
    }[dt_tag]
    mm = lambda ap: ap  # noqa: E731

    nc = bacc.Bacc("TRN2", target_bir_lowering=False, debug=False)
    xT = nc.dram_tensor("xT", [P, KO, C], dt_in, kind="ExternalInput").ap()
    w1 = nc.dram_tensor("w1", [P, KO, F], dt_in, kind="ExternalInput").ap()
    b1 = nc.dram_tensor("b1", [P, FO], f32, kind="ExternalInput").ap()
    w2 = nc.dram_tensor("w2", [P, FO, H], dt_in, kind="ExternalInput").ap()
    b2 = nc.dram_tensor("b2", [P, HO], f32, kind="ExternalInput").ap()
    yT = nc.dram_tensor("yT", [P, HO, C], f32, kind="ExternalOutput").ap()
    chunks = _chunks(C)

    with tile.TileContext(nc) as tc:
        with (
            tc.tile_pool(name="const", bufs=1) as const,
            tc.tile_pool(name="w1p", bufs=3) as w1p,
            tc.tile_pool(name="w2p", bufs=3) as w2p,
            tc.tile_pool(name="hp", bufs=1) as hp,
            tc.tile_pool(name="yp", bufs=3) as yp,
            tc.tile_pool(name="ps1", bufs=2, space="PSUM") as ps1,
            tc.tile_pool(name="ps2", bufs=6, space="PSUM") as ps2,
        ):
            xT_sb = const.tile([P, KO, C], dt_in)
            nc.sync.dma_start(xT_sb[:], xT[:])
            b1_sb = const.tile([P, FO], f32)
            nc.sync.dma_start(b1_sb[:], b1[:])
            b2_sb = const.tile([P, HO], f32)
            nc.sync.dma_start(b2_sb[:], b2[:])
            hT = hp.tile([P, FO, C], dt_in)

            # phase 1: hT = gelu(w1^T-stationary matmul + b1)
            for fg in range(FO // 2):  # f-groups of 256 cols
                w1t = w1p.tile([P, KO, 2 * P], dt_in, tag="w1t")
                nc.sync.dma_start(w1t[:], w1[:, :, fg * 2 * P : (fg + 1) * 2 * P])
                for fl in range(2):
                    fo = fg * 2 + fl
                    for c0, cw in chunks:
                        ps = ps1.tile([P, 512], f32, tag="ps1")
                        for ko in range(KO):
                            nc.tensor.matmul(
                                ps[:, :cw],
                                mm(w1t[:, ko, fl * P : (fl + 1) * P]),
                                mm(xT_sb[:, ko, c0 : c0 + cw]),
                                start=(ko == 0),
                                stop=(ko == KO - 1),
                            )
                        nc.scalar.activation(
                            hT[:, fo, c0 : c0 + cw],
                            ps[:, :cw],
                            mybir.ActivationFunctionType.Gelu,
                            bias=b1_sb[:, fo : fo + 1],
                        )

            # phase 2: yT = w2^T-stationary matmul over hT + b2
            for hg in range(HO // 2):  # h-groups of 256 cols
                pss = {}
                for hl in range(2):
                    for ci in range(len(chunks)):
                        pss[(hl, ci)] = ps2.tile([P, 512], f32, tag="ps2", name="ps2t")
                for fq in range(4):
                    w2t = w2p.tile([P, 8, 2 * P], dt_in, tag="w2t")
                    nc.sync.dma_start(
                        w2t[:],
                        w2[:, fq * 8 : (fq + 1) * 8, hg * 2 * P : (hg + 1) * 2 * P],
                    )
                    for fl in range(8):
                        fo = fq * 8 + fl
                        for hl in range(2):
                            for ci, (c0, cw) in enumerate(chunks):
                                nc.tensor.matmul(
                                    pss[(hl, ci)][:, :cw],
                                    mm(w2t[:, fl, hl * P : (hl + 1) * P]),
                                    mm(hT[:, fo, c0 : c0 + cw]),
                                    start=(fo == 0),
                                    stop=(fo == FO - 1),
                                )
                for hl in range(2):
                    ho = hg * 2 + hl
                    for ci, (c0, cw) in enumerate(chunks):
                        ysb = yp.tile([P, 512], f32, tag="ysb")
                        nc.scalar.activation(
                            ysb[:, :cw],
                            pss[(hl, ci)][:, :cw],
                            mybir.ActivationFunctionType.Identity,
                            bias=b2_sb[:, ho : ho + 1],
                        )
                        nc.sync.dma_start(yT[:, ho, c0 : c0 + cw], ysb[:, :cw])
    nc.compile()
    return nc


def _get_nc(C, dt_tag):
    key = (C, dt_tag)
    if key not in _nc_cache:
        _nc_cache[key] = _build(C, dt_tag)
    return _nc_cache[key]


def _route(x, router_w):
    """Top-2 routing identical (up to fp noise far below the tie margin)
    to jax.lax.top_k + softmax in the reference."""
    n = x.shape[0]
    logits = x.astype(np.float64) @ router_w.astype(np.float64)
    r = np.arange(n)
    i1 = np.argmax(logits, 1)
    masked = logits.copy()
    masked[r, i1] = -np.inf
    i2 = np.argmax(masked, 1)
    tl = np.stack([logits[r, i1], logits[r, i2]], 1).astype(np.float32)
    e = np.exp(tl - tl.max(1, keepdims=True))
    s = (e / e.sum(1, keepdims=True)).astype(np.float32)
    return np.stack([i1, i2], 1), s


def _prepare(inputs, dt_tag):
    hs = np.asarray(inputs["hidden_states"], np.float32)
    router_w = np.asarray(inputs["router_w"], np.float32)
    w1 = np.asarray(inputs["w1"], np.float32)
    b1 = np.asarray(inputs["b1"], np.float32)
    w2 = np.asarray(inputs["w2"], np.float32)
    b2 = np.asarray(inputs["b2"], np.float32)
    S, B, H_ = hs.shape
    x = hs.reshape(S * B, H_)

    idx2, scores = _route(x, router_w)
    tok = [np.flatnonzero((idx2 == e).any(1)) for e in range(E)]
    wts = []
    for e in range(E):
        sel = idx2[tok[e]] == e  # [n_e, 2]; exactly one True per row
        wts.append(
            np.where(sel[:, 0], scores[tok[e], 0], scores[tok[e], 1]).astype(
                np.float32
            )
        )

    maxn = max(len(t) for t in tok)
    C = max(256, ((maxn + 63) // 64) * 64)

    if dt_tag == "bf16":
        import ml_dtypes

        np_in = ml_dtypes.bfloat16
    else:
        np_in = np.float32

    in_maps = []
    for e in range(E):
        n_e = len(tok[e])
        xT = np.zeros((P, KO, C), np_in)
        xT[:, :, :n_e] = x[tok[e]].T.reshape(KO, P, n_e).transpose(1, 0, 2)
        in_maps.append(
            {
                "xT": xT,
                "w1": np.ascontiguousarray(
                    w1[e].reshape(KO, P, F).transpose(1, 0, 2).astype(np_in)
                ),
                "b1": np.ascontiguousarray(b1[e].reshape(FO, P).T),
                "w2": np.ascontiguousarray(
                    w2[e].reshape(FO, P, H_).transpose(1, 0, 2).astype(np_in)
                ),
                "b2": np.ascontiguousarray(b2[e].reshape(HO, P).T),
            }
        )
    return (S, B, H_), x, tok, wts, C, in_maps


def _combine(shape, tok, wts, results):
    S, B, H_ = shape
    out = np.zeros((S * B, H_), np.float32)
    for e in range(E):
        n_e = len(tok[e])
        yT = results[e]["yT"]  # [P, HO, C] f32
        y = yT.transpose(1, 0, 2).reshape(H_, -1)[:, :n_e].T
        out[tok[e]] += wts[e][:, None] * y
    return out.reshape(S, B, H_)


def kernel(**inputs):
    dt_tag = KERNEL_DTYPE
    shape, _x, tok, wts, C, in_maps = _prepare(inputs, dt_tag)
    nc = _get_nc(C, dt_tag)
    res = run_bass_kernel_spmd(nc, in_maps, core_ids=list(range(NCORES)))
    return _combine(shape, tok, wts, res.results)



# revision 2
# speedup vs baseline: 1.1472x; 1.1472x over previous
"""MoE routed-MLP (GPTNeoX) Trainium2 kernel.

Expert-parallel over 8 NeuronCores: core e holds expert e's weights.
Host computes the (tiny) router + top-2 dispatch, gathers each expert's
tokens into a padded batch, and scatter-adds the weighted expert outputs
back. Each core runs the same SPMD Bass program:

    hT[f, c] = gelu( sum_k w1[k, f] * xT[k, c] + b1[f] )      (f on partitions)
    yT[h, c] = sum_f w2[f, h] * hT[f, c] + b2[h]              (h on partitions)

Both matmuls keep the weights as the stationary operand so the
intermediate never needs an on-chip transpose. Weights are streamed once
into SBUF (they fit entirely at 16-bit) with a handful of large
contiguous DMAs ordered so the PE can start as soon as the first w1
slice lands; everything else overlaps behind the matmul stream.
"""

import numpy as np

import concourse.bass as bass  # noqa: F401  (bass types used via tile/bacc)
import concourse.mybir as mybir
import concourse.tile as tile
from concourse import bacc
from concourse.bass_utils import run_bass_kernel_spmd

H = 1024
F = 4096
E = 8
NCORES = 8
P = 128
KO = H // P  # 8   k-chunks for the H contraction
FO = F // P  # 32  f-tiles
HO = H // P  # 8   h-tiles

# "fp16" | "bf16" | "fp32r" | "fp32"
KERNEL_DTYPE = "fp16"

_nc_cache = {}


def _chunks(C):
    """Split [0, C) into column chunks, each <= 512 (one PSUM bank of f32),
    as balanced as possible in multiples of 8."""
    n = (C + 511) // 512
    base = (C // n) // 8 * 8
    widths = [base] * n
    rem = C - base * n
    i = 0
    while rem > 0:
        widths[i] += 8
        rem -= 8
        i = (i + 1) % n
    out, off = [], 0
    for w in widths:
        out.append((off, w))
        off += w
    return out


def _build(C, dt_tag):
    f32 = mybir.dt.float32
    dt_in = {
        "fp16": mybir.dt.float16,
        "bf16": mybir.dt.bfloat16,
        "fp32r": mybir.dt.float32r,
        "fp32": f32,
    }[dt_tag]

    nc = bacc.Bacc("TRN2", target_bir_lowering=False, debug=False)
    xT = nc.dram_tensor("xT", [P, KO, C], dt_in, kind="ExternalInput").ap()
    w1 = nc.dram_tensor("w1", [P, KO, F], dt_in, kind="ExternalInput").ap()
    b1 = nc.dram_tensor("b1", [P, FO], f32, kind="ExternalInput").ap()
    w2 = nc.dram_tensor("w2", [P, FO, H], dt_in, kind="ExternalInput").ap()
    b2 = nc.dram_tensor("b2", [P, HO], f32, kind="ExternalInput").ap()
    yT = nc.dram_tensor("yT", [P, HO, C], f32, kind="ExternalOutput").ap()
    chunks = _chunks(C)

    with tile.TileContext(nc) as tc:
        with (
            tc.tile_pool(name="const", bufs=1) as const,
            tc.tile_pool(name="yp", bufs=4) as yp,
            tc.tile_pool(name="ps1", bufs=3, space="PSUM") as ps1,
            tc.tile_pool(name="ps2", bufs=4, space="PSUM") as ps2,
        ):
            b1_sb = const.tile([P, FO], f32)
            nc.sync.dma_start(b1_sb[:], b1[:])
            b2_sb = const.tile([P, HO], f32)
            nc.sync.dma_start(b2_sb[:], b2[:])

            # x, then w1 (in 512-col slices so the first matmuls gate on
            # only ~1/8 of it), then w2 — all resident in SBUF.
            xT_sb = const.tile([P, KO, C], dt_in)
            c_half = (len(chunks) + 1) // 2
            c_mid = chunks[c_half - 1][0] + chunks[c_half - 1][1]
            nc.sync.dma_start(xT_sb[:, :, :c_mid], xT[:, :, :c_mid])
            w1_sb = const.tile([P, KO, F], dt_in)
            nc.sync.dma_start(w1_sb[:, :, :512], w1[:, :, :512])
            if c_mid < C:
                nc.sync.dma_start(xT_sb[:, :, c_mid:], xT[:, :, c_mid:])
            for j in range(1, F // 512):
                nc.sync.dma_start(
                    w1_sb[:, :, j * 512 : (j + 1) * 512],
                    w1[:, :, j * 512 : (j + 1) * 512],
                )
            w2_sb = const.tile([P, FO, H], dt_in)
            for q in range(4):
                nc.sync.dma_start(
                    w2_sb[:, q * 8 : (q + 1) * 8, :], w2[:, q * 8 : (q + 1) * 8, :]
                )

            hT = const.tile([P, FO, C], dt_in)

            # phase 1: hT = gelu(w1^T-stationary matmul + b1)
            for fo in range(FO):
                for c0, cw in chunks:
                    ps = ps1.tile([P, 512], f32, tag="ps1")
                    for ko in range(KO):
                        nc.tensor.matmul(
                            ps[:, :cw],
                            w1_sb[:, ko, fo * P : (fo + 1) * P],
                            xT_sb[:, ko, c0 : c0 + cw],
                            start=(ko == 0),
                            stop=(ko == KO - 1),
                        )
                    nc.scalar.activation(
                        hT[:, fo, c0 : c0 + cw],
                        ps[:, :cw],
                        mybir.ActivationFunctionType.Gelu,
                        bias=b1_sb[:, fo : fo + 1],
                    )

            # phase 2: yT = w2^T-stationary matmul over hT + b2
            for ho in range(HO):
                for c0, cw in chunks:
                    ps = ps2.tile([P, 512], f32, tag="ps2")
                    for fo in range(FO):
                        nc.tensor.matmul(
                            ps[:, :cw],
                            w2_sb[:, fo, ho * P : (ho + 1) * P],
                            hT[:, fo, c0 : c0 + cw],
                            start=(fo == 0),
                            stop=(fo == FO - 1),
                        )
                    ysb = yp.tile([P, 512], f32, tag="ysb")
                    nc.scalar.activation(
                        ysb[:, :cw],
                        ps[:, :cw],
                        mybir.ActivationFunctionType.Identity,
                        bias=b2_sb[:, ho : ho + 1],
                    )
                    nc.sync.dma_start(yT[:, ho, c0 : c0 + cw], ysb[:, :cw])
    nc.compile()
    return nc


def _get_nc(C, dt_tag):
    key = (C, dt_tag)
    if key not in _nc_cache:
        _nc_cache[key] = _build(C, dt_tag)
    return _nc_cache[key]


def _route(x, router_w):
    """Top-2 routing identical (up to fp noise far below the tie margin)
    to jax.lax.top_k + softmax in the reference."""
    n = x.shape[0]
    logits = x.astype(np.float64) @ router_w.astype(np.float64)
    r = np.arange(n)
    i1 = np.argmax(logits, 1)
    masked = logits.copy()
    masked[r, i1] = -np.inf
    i2 = np.argmax(masked, 1)
    tl = np.stack([logits[r, i1], logits[r, i2]], 1).astype(np.float32)
    e = np.exp(tl - tl.max(1, keepdims=True))
    s = (e / e.sum(1, keepdims=True)).astype(np.float32)
    return np.stack([i1, i2], 1), s


def _np_dtype(dt_tag):
    if dt_tag == "bf16":
        import ml_dtypes

        return ml_dtypes.bfloat16
    if dt_tag == "fp16":
        return np.float16
    return np.float32


def _prepare(inputs, dt_tag):
    hs = np.asarray(inputs["hidden_states"], np.float32)
    router_w = np.asarray(inputs["router_w"], np.float32)
    w1 = np.asarray(inputs["w1"], np.float32)
    b1 = np.asarray(inputs["b1"], np.float32)
    w2 = np.asarray(inputs["w2"], np.float32)
    b2 = np.asarray(inputs["b2"], np.float32)
    S, B, H_ = hs.shape
    x = hs.reshape(S * B, H_)

    idx2, scores = _route(x, router_w)
    tok = [np.flatnonzero((idx2 == e).any(1)) for e in range(E)]
    wts = []
    for e in range(E):
        sel = idx2[tok[e]] == e  # [n_e, 2]; exactly one True per row
        wts.append(
            np.where(sel[:, 0], scores[tok[e], 0], scores[tok[e], 1]).astype(
                np.float32
            )
        )

    maxn = max(len(t) for t in tok)
    C = max(64, ((maxn + 7) // 8) * 8)

    np_in = _np_dtype(dt_tag)

    in_maps = []
    for e in range(E):
        n_e = len(tok[e])
        xT = np.zeros((P, KO, C), np_in)
        xT[:, :, :n_e] = x[tok[e]].T.reshape(KO, P, n_e).transpose(1, 0, 2)
        in_maps.append(
            {
                "xT": xT,
                "w1": np.ascontiguousarray(
                    w1[e].reshape(KO, P, F).transpose(1, 0, 2).astype(np_in)
                ),
                "b1": np.ascontiguousarray(b1[e].reshape(FO, P).T),
                "w2": np.ascontiguousarray(
                    w2[e].reshape(FO, P, H_).transpose(1, 0, 2).astype(np_in)
                ),
                "b2": np.ascontiguousarray(b2[e].reshape(HO, P).T),
            }
        )
    return (S, B, H_), x, tok, wts, C, in_maps


def _combine(shape, tok, wts, results):
    S, B, H_ = shape
    out = np.zeros((S * B, H_), np.float32)
    for e in range(E):
        n_e = len(tok[e])
        yT = results[e]["yT"]  # [P, HO, C] f32
        y = yT.transpose(1, 0, 2).reshape(H_, -1)[:, :n_e].T
        out[tok[e]] += wts[e][:, None] * y
    return out.reshape(S, B, H_)


def kernel(**inputs):
    dt_tag = KERNEL_DTYPE
    shape, _x, tok, wts, C, in_maps = _prepare(inputs, dt_tag)
    nc = _get_nc(C, dt_tag)
    res = run_bass_kernel_spmd(nc, in_maps, core_ids=list(range(NCORES)))
    return _combine(shape, tok, wts, res.results)
